# revision 22
# baseline (speedup 1.0000x reference)
"""Trainium2 Bass kernel for causal GQA self-attention (B=2, T=2048, C=2048,
Hq=16, Hkv=4, d=128, RoPE base 1e6).

Sharding: 8 cores = 2 batches x 4 kv-head groups. Each core computes, for its
(batch b, kv group g): the q/k/v projections restricted to that group (4 q
heads + 1 kv head), RoPE, causal attention, and the partial o_proj
(y_group @ Wo[group rows]). The host sums the 4 partial o_proj outputs per
batch (the all-reduce/unshard step of tensor parallelism).

Device layout notes:
  - x is shipped pre-transposed (xT = x[b].T) and pre-cast to bf16 so the
    contraction dim (C) lands on SBUF partitions for all projection matmuls.
    xT is DMA'd t4-major (512-col t-chunks) so the first projection
    accumulation chain starts ~7us in instead of waiting for the full 8.4MB.
  - q/k are produced transposed (qT/kT = [d, T]); scores are computed
    transposed (S^T = k @ qT, [k, q]) so softmax exp needs no cross-partition
    work and P^T feeds the AV matmul directly (yT = v^T @ P^T), no transposes.
  - projections and attention heads are interleaved (k, q0, v, head0, q1,
    head1, ...) so ACT-engine exp work spreads across the whole kernel
    instead of piling up after all projections.
  - diagonal score/rowsum matmul streams and exp ranges are trimmed to the
    causal lower-triangular region (block granularity).
  - softmax skips max-subtraction (scores are O(1); exp cannot overflow) and
    normalizes at the end: rowsums via a ones-vector matmul, 1/s via DVE
    reciprocal, broadcast via a DRAM round-trip DMA, applied after an eager
    unnormalized PSUM->SBUF eviction of yT.
  - the v bias is folded out entirely: since softmax rows sum to 1, bv
    contributes the constant row bv_tiled @ Wo_g, added on the host.
  - o_proj runs as a dependency-free tail block (all yTn ready), output is
    stored in bf16 to halve the 16.8MB store traffic.
  - DMA issues are spread across engine queues: xT/sscr/stores on Sync,
    weights on ACT, tables on DVE, rope partition-shifts on idle GpSimd.
"""

import numpy as np
import ml_dtypes

import concourse.bass as bass
import concourse.mybir as mybir
from concourse import bacc
from concourse.tile import TileContext
from concourse.bass_utils import run_bass_kernel_spmd
from concourse.masks import make_identity

BF16 = mybir.dt.bfloat16
F32 = mybir.dt.float32

T = 2048
C = 2048
D = 128
NH = 4           # q heads per core
CI = C // 128    # contraction chunks
TC = T // 512    # t chunks of 512
TB = T // 128    # t blocks of 128
SCALE = 1.0 / np.sqrt(D)

_PROGRAM = None


def _ts(i, s):
    return bass.ts(i, s)


def _patch_act_tables():
    """Force every ACT function this kernel uses to resolve to the
    natural_log_exp_and_others table set, so the whole kernel needs exactly
    one ACT_TABLE_LOAD. Returns an undo callable."""
    import concourse.bacc as bacc_mod

    orig = bacc_mod.get_activation_tables
    A = mybir.ActivationFunctionType
    mine = {A.Exp, A.Ln, A.Identity, A.Copy}

    def patched(arch):
        tables = dict(orig(arch))
        for name in tables:
            if name != "natural_log_exp_and_others":
                tables[name] = set(tables[name]) - mine
        return tables

    bacc_mod.get_activation_tables = patched

    def undo():
        bacc_mod.get_activation_tables = orig

    return undo


def _build_program():
    undo = _patch_act_tables()
    try:
        return _build_program_inner()
    finally:
        undo()


def _build_program_inner():
    nc = bacc.Bacc("TRN2", target_bir_lowering=False, debug=False, num_devices=8)

    xT_d = nc.dram_tensor("xT", [C, T], BF16, kind="ExternalInput").ap()
    wq_d = nc.dram_tensor("wq", [C, NH * D], BF16, kind="ExternalInput").ap()
    wk_d = nc.dram_tensor("wk", [C, D], BF16, kind="ExternalInput").ap()
    wv_d = nc.dram_tensor("wv", [C, D], BF16, kind="ExternalInput").ap()
    wo_d = nc.dram_tensor("wo", [NH * D, C], BF16, kind="ExternalInput").ap()
    bq_d = nc.dram_tensor("bq", [D, NH], F32, kind="ExternalInput").ap()
    bk_d = nc.dram_tensor("bk", [D, 1], F32, kind="ExternalInput").ap()
    cos_d = nc.dram_tensor("cosT", [D, T], BF16, kind="ExternalInput").ap()
    sin_d = nc.dram_tensor("sinT", [D, T], BF16, kind="ExternalInput").ap()
    tri_d = nc.dram_tensor("tri", [D, D], BF16, kind="ExternalInput").ap()
    out_d = nc.dram_tensor("out", [T, C], BF16, kind="ExternalOutput").ap()
    # scratch for the per-(head, q-chunk) 1/rowsum broadcast round-trip
    sscr = nc.dram_tensor("sscr", [NH * TC, 512], F32).ap()

    Ident = mybir.ActivationFunctionType.Identity
    Exp = mybir.ActivationFunctionType.Exp

    with TileContext(nc) as tc:
        with (
            tc.tile_pool(name="consts", bufs=1) as consts,
            tc.tile_pool(name="acts", bufs=1) as acts,
        ):
            # ---- resident constants -------------------------------------
            xT_sb = consts.tile([128, CI, T], BF16)
            wq_sb = consts.tile([128, CI, NH * D], BF16)
            wk_sb = consts.tile([128, CI, D], BF16)
            wv_sb = consts.tile([128, CI, D], BF16)
            wo_sb = consts.tile([128, NH, C], BF16)
            bq_sb = consts.tile([128, NH], F32)
            bk_sb = consts.tile([128, 1], F32)
            cos_sb = consts.tile([128, T], BF16)
            sin_sb = consts.tile([128, T], BF16)
            tri_sb = consts.tile([128, 128], BF16)
            ones_sb = consts.tile([128, 1], BF16)
            ident_sb = consts.tile([128, 128], BF16)

            # xT on the Sync queue, t4-major so the first proj chain starts
            # as soon as the first 2.1MB lands. The first 4 chunks are split
            # across partitions into 4 DMAs each, landing on parallel DMA
            # engines, so the very first matmul starts ~3us in, not ~7us.
            for ci in range(4):
                for p4 in range(4):
                    nc.sync.dma_start(
                        out=xT_sb[p4 * 32 : (p4 + 1) * 32, ci, 0:512],
                        in_=xT_d[ci * 128 + p4 * 32 : ci * 128 + (p4 + 1) * 32, 0:512],
                    )
            for t4 in range(TC):
                for ci in range(CI):
                    if t4 == 0 and ci < 4:
                        continue
                    nc.sync.dma_start(
                        out=xT_sb[:, ci, _ts(t4, 512)],
                        in_=xT_d[_ts(ci, 128), _ts(t4, 512)],
                    )
            # k/v weights + biases on the ACT queue (needed first).
            for g4 in range(4):
                wkg = wk_d[g4 * 512 : (g4 + 1) * 512, :]
                nc.scalar.dma_start(
                    out=wk_sb[:, g4 * 4 : (g4 + 1) * 4, :],
                    in_=bass.AP(
                        tensor=wkg.tensor,
                        offset=wkg.offset,
                        ap=[[D, 128], [128 * D, 4], [1, D]],
                    ),
                )
                wvg = wv_d[g4 * 512 : (g4 + 1) * 512, :]
                nc.scalar.dma_start(
                    out=wv_sb[:, g4 * 4 : (g4 + 1) * 4, :],
                    in_=bass.AP(
                        tensor=wvg.tensor,
                        offset=wvg.offset,
                        ap=[[D, 128], [128 * D, 4], [1, D]],
                    ),
                )
            nc.scalar.dma_start(out=bq_sb[:], in_=bq_d[:])
            nc.scalar.dma_start(out=bk_sb[:], in_=bk_d[:])
            # tables + q weights also on the ACT queue, ordered by first use
            # (all issued before the first bias op is queued); wq last, it is
            # not needed until the third chain of the first t4 block
            nc.scalar.dma_start(out=cos_sb[:], in_=cos_d[:])
            nc.scalar.dma_start(out=sin_sb[:], in_=sin_d[:])
            nc.scalar.dma_start(out=tri_sb[:], in_=tri_d[:])
            # wq on the GpSimd queue: its SWDGE issues are slower but rope
            # shifts (the only later GpSimd work) are not needed until the
            # first head, and this keeps the ACT queue free for biases
            for ci in range(CI):
                nc.gpsimd.dma_start(out=wq_sb[:, ci, :], in_=wq_d[_ts(ci, 128), :])
            nc.vector.memset(ones_sb[:], 1.0)
            make_identity(nc, ident_sb[:])

            # ---- persistent activations ---------------------------------
            qT_all = acts.tile([128, NH, T], BF16)   # rotated q^T per head
            kT_all = acts.tile([128, T], BF16)       # rotated k^T
            v_sb = acts.tile([128, TB, D], BF16)     # v in natural [t, d] blocks
            yTn = acts.tile([128, NH, T], BF16)      # normalized y^T per head

            with tc.tile_pool(name="rope", bufs=2) as rope_pool:
                def emit_evict(m, t4, ps, vtp=None):
                    if m == 5:
                        # v: no bias on device (bv folded out on host)
                        vbb = rope_pool.tile([128, 512], BF16, tag="vbb")
                        nc.vector.tensor_copy(vbb[:], ps[:])
                        for j in range(4):
                            tb = t4 * 4 + j
                            pt = vtp.tile([128, 128], BF16)
                            nc.tensor.transpose(
                                pt[:], vbb[:, _ts(j, 128)], ident_sb[:]
                            )
                            nc.vector.tensor_copy(v_sb[:, tb, :], pt[:])
                    else:
                        bias_ap = bq_sb[:, m : m + 1] if m < 4 else bk_sb[:, 0:1]
                        qb = rope_pool.tile([128, 512], BF16, tag="qb")
                        if m == 0:
                            # spread PSUM evictions across engines: the four
                            # zipper chains stop nearly together, and three
                            # serialized ACT biases would stall the pool
                            nc.vector.tensor_scalar_add(qb[:], ps[:], bias_ap)
                        else:
                            nc.scalar.activation(qb[:], ps[:], Ident, bias=bias_ap)
                        # rotate-half partition shift on the idle GpSimd queue
                        sh = rope_pool.tile([128, 512], BF16, tag="sh")
                        nc.gpsimd.dma_start(out=sh[0:64, :], in_=qb[64:128, :])
                        nc.gpsimd.dma_start(out=sh[64:128, :], in_=qb[0:64, :])
                        t1 = rope_pool.tile([128, 512], BF16, tag="t1")
                        nc.vector.tensor_mul(t1[:], qb[:], cos_sb[:, _ts(t4, 512)])
                        nc.vector.tensor_mul(sh[:], sh[:], sin_sb[:, _ts(t4, 512)])
                        dest = (
                            qT_all[:, m, _ts(t4, 512)]
                            if m < 4
                            else kT_all[:, _ts(t4, 512)]
                        )
                        nc.vector.tensor_add(dest, t1[:], sh[:])

                def w_of(m, ci):
                    if m < 4:
                        return wq_sb[:, ci, _ts(m, 128)]
                    if m == 4:
                        return wk_sb[:, ci, :]
                    return wv_sb[:, ci, :]

                with (
                    tc.tile_pool(name="pp4", bufs=6, space="PSUM") as pp4,
                    tc.tile_pool(name="vtp", bufs=2, space="PSUM") as vtp,
                ):
                    # rolling 5-lane zipper: five accumulation chains advance
                    # together, one matmul per freshly-landed xT ci chunk per
                    # lane, with lane starts skewed so chain stops (and their
                    # PSUM evictions) stagger instead of bunching at t4
                    # boundaries
                    seq = [(m, t4) for t4 in range(TC) for m in (4, 5, 0, 1, 2)]
                    NL = 5
                    lane_chains = [seq[i::NL] for i in range(NL)]
                    delays = [3 * i for i in range(NL)]
                    pos = [0] * NL
                    ci_pos = [0] * NL
                    cur = [None] * NL
                    done, step = 0, 0
                    while done < len(seq):
                        for i in range(NL):
                            if step < delays[i] or pos[i] >= len(lane_chains[i]):
                                continue
                            m, t4 = lane_chains[i][pos[i]]
                            if cur[i] is None:
                                cur[i] = pp4.tile(
                                    [128, 512], F32, name=f"ps{m}_{t4}", tag="ps"
                                )
                            ci = ci_pos[i]
                            nc.tensor.matmul(
                                cur[i][:],
                                w_of(m, ci),
                                xT_sb[:, ci, _ts(t4, 512)],
                                start=(ci == 0),
                                stop=(ci == CI - 1),
                            )
                            ci_pos[i] += 1
                            if ci_pos[i] == CI:
                                emit_evict(m, t4, cur[i], vtp)
                                cur[i] = None
                                ci_pos[i] = 0
                                pos[i] += 1
                                done += 1
                        step += 1

                with (
                    tc.tile_pool(name="pp", bufs=2, space="PSUM") as pp,
                    tc.tile_pool(name="st", bufs=3, space="PSUM") as stp,
                    tc.tile_pool(name="yt", bufs=2, space="PSUM") as ytp,
                    tc.tile_pool(name="rs", bufs=1, space="PSUM") as rsp,
                    tc.tile_pool(name="ptp", bufs=6) as ptp,
                    tc.tile_pool(name="sivb", bufs=2) as sivb,
                    tc.tile_pool(name="sip", bufs=2) as sip,
                    tc.tile_pool(name="ytu", bufs=4) as ytup,
                ):
                    def emit_proj(m):
                        for t4 in range(TC):
                            ps = pp.tile([128, 512], F32)
                            for ci in range(CI):
                                nc.tensor.matmul(
                                    ps[:],
                                    w_of(m, ci),
                                    xT_sb[:, ci, _ts(t4, 512)],
                                    start=(ci == 0),
                                    stop=(ci == CI - 1),
                                )
                            emit_evict(m, t4, ps)

                    def emit_head(h):
                        for qc in (3, 2, 1, 0):
                            yt_ps = ytp.tile([128, 512], F32)
                            rs_ps = rsp.tile([1, 512], F32)
                            nkb = 4 * (qc + 1)
                            for kb in range(nkb):
                                j = kb - 4 * qc  # >=0 on the diagonal group
                                lo = j * 128 if j > 0 else 0
                                st_ps = stp.tile([128, 512], F32)
                                # scores, trimmed to the causal region
                                nc.tensor.matmul(
                                    st_ps[:, lo:512],
                                    kT_all[:, _ts(kb, 128)],
                                    qT_all[:, h, qc * 512 + lo : qc * 512 + 512],
                                    start=True,
                                    stop=True,
                                )
                                pt = ptp.tile([128, 512], BF16, tag="pt")
                                nc.scalar.activation(
                                    pt[:, lo:512], st_ps[:, lo:512], Exp,
                                    scale=SCALE,
                                )
                                if j >= 0:
                                    if j > 0:
                                        nc.vector.memset(pt[:, 0:lo], 0.0)
                                    nc.vector.tensor_mul(
                                        pt[:, lo : lo + 128],
                                        pt[:, lo : lo + 128],
                                        tri_sb[:],
                                    )
                                nc.tensor.matmul(
                                    yt_ps[:],
                                    v_sb[:, kb, :],
                                    pt[:],
                                    start=(kb == 0),
                                    stop=(kb == nkb - 1),
                                )
                                # rowsums: trimmed, except the last block
                                # streams full width so every PSUM region
                                # sees its stop flag
                                rlo = lo if j < 3 else 0
                                nc.tensor.matmul(
                                    rs_ps[0:1, rlo:512],
                                    ones_sb[:],
                                    pt[:, rlo:512],
                                    start=(kb == 0),
                                    stop=(kb == nkb - 1),
                                )
                            # evict yT unnormalized right away (frees the
                            # PSUM bank without waiting on the 1/s chain)
                            ytu = ytup.tile([128, 512], BF16)
                            nc.vector.tensor_copy(ytu[:], yt_ps[:])
                            # 1/s on DVE, then DRAM round-trip broadcast
                            idx = h * TC + qc
                            si = sip.tile([1, 512], F32, tag="si")
                            nc.vector.reciprocal_approx_fast(si[:], rs_ps[:])
                            nc.sync.dma_start(out=sscr[idx : idx + 1, :], in_=si[:])
                            sb = sivb.tile([128, 512], F32)
                            row = sscr[idx : idx + 1, :]
                            bc = bass.AP(
                                tensor=row.tensor,
                                offset=row.offset,
                                ap=[[0, 128]] + row.ap[1:],
                            )
                            nc.sync.dma_start(out=sb[:], in_=bc)
                            # normalize on the idle GpSimd engine: a DVE mul
                            # here would stall the in-order DVE queue on the
                            # broadcast round-trip, delaying the next chunk's
                            # mask work and stalling PE
                            nc.gpsimd.tensor_mul(
                                yTn[:, h, _ts(qc, 512)], ytu[:], sb[:]
                            )

                    emit_head(0)
                    # wo on the ACT queue once attention is underway
                    for hh in range(NH):
                        nc.scalar.dma_start(
                            out=wo_sb[:, hh, :], in_=wo_d[_ts(hh, 128), :]
                        )
                    emit_head(1)
                    emit_proj(3)
                    emit_head(2)
                    emit_head(3)

            # ---- o_proj tail: all yTn ready, pure streaming -------------
            with (
                tc.tile_pool(name="po", bufs=6, space="PSUM") as pop,
                tc.tile_pool(name="oe", bufs=6) as oep,
            ):
                # qc-descending ti order: yTn for low qc finishes last, so
                # the first o_proj tiles must not depend on it
                for qcg in (3, 2, 1, 0):
                    for ti in range(4 * qcg, 4 * qcg + 4):
                        for nj in range(TC):
                            ps = pop.tile([128, 512], F32)
                            for h in range(NH):
                                nc.tensor.matmul(
                                    ps[:],
                                    yTn[:, h, _ts(ti, 128)],
                                    wo_sb[:, h, _ts(nj, 512)],
                                    start=(h == 0),
                                    stop=(h == NH - 1),
                                )
                            oe = oep.tile([128, 512], BF16)
                            if (ti * TC + nj) % 2 == 0:
                                nc.vector.tensor_copy(oe[:], ps[:])
                                nc.sync.dma_start(
                                    out=out_d[_ts(ti, 128), _ts(nj, 512)],
                                    in_=oe[:],
                                )
                            else:
                                nc.scalar.copy(oe[:], ps[:])
                                nc.scalar.dma_start(
                                    out=out_d[_ts(ti, 128), _ts(nj, 512)],
                                    in_=oe[:],
                                )

    nc.finalize()
    return nc


def _get_program():
    global _PROGRAM
    if _PROGRAM is None:
        _PROGRAM = _build_program()
    return _PROGRAM


def _rope_tables():
    inv_freq = 1.0 / (1000000.0 ** (np.arange(0, D, 2, dtype=np.float64) / D))
    pos = np.arange(T, dtype=np.float64)
    si = np.outer(pos, inv_freq)                      # [T, D/2]
    cos_h, sin_h = np.cos(si), np.sin(si)
    cos = np.stack([cos_h, cos_h], axis=-1).reshape(T, D)
    sin = np.stack([sin_h, sin_h], axis=-1).reshape(T, D)
    cosT = np.ascontiguousarray(cos.T).astype(np.float32)   # [D, T]
    sinT = np.ascontiguousarray(sin.T).astype(np.float32)
    # rotate-half as a partition shift: sh[i<64]=q[i+64], sh[i>=64]=q[i-64];
    # q_rot = q*cos + sh*sin_signed with the -1 for i<64 baked into the table
    sinT[: D // 2] *= -1.0
    return cosT, sinT


def make_in_maps(x, Wq, bq, Wk, bk, Wv, bv, Wo):
    bf = ml_dtypes.bfloat16
    cosT, sinT = _rope_tables()
    tri = np.triu(np.ones((D, D), dtype=np.float32)).astype(bf)  # [k, q]: q >= k
    in_maps = []
    for b in range(2):
        xT = np.ascontiguousarray(x[b].T).astype(bf)
        for g in range(4):
            in_maps.append(
                {
                    "xT": xT,
                    "wq": np.ascontiguousarray(Wq[:, g * 512 : (g + 1) * 512]).astype(bf),
                    "wk": np.ascontiguousarray(Wk[:, g * 128 : (g + 1) * 128]).astype(bf),
                    "wv": np.ascontiguousarray(Wv[:, g * 128 : (g + 1) * 128]).astype(bf),
                    "wo": np.ascontiguousarray(Wo[g * 512 : (g + 1) * 512, :]).astype(bf),
                    "bq": np.ascontiguousarray(
                        bq[g * 512 : (g + 1) * 512].reshape(NH, D).T
                    ).astype(np.float32),
                    "bk": np.ascontiguousarray(
                        bk[g * 128 : (g + 1) * 128].reshape(D, 1)
                    ).astype(np.float32),
                    "cosT": cosT.astype(bf),
                    "sinT": sinT.astype(bf),
                    "tri": tri,
                }
            )
    return in_maps


def combine_outputs(res, inputs):
    bv, Wo = np.asarray(inputs["bv"]), np.asarray(inputs["Wo"])
    out = np.zeros((2, T, C), dtype=np.float32)
    for c in range(8):
        g = c % 4
        out[c // 4] += np.asarray(res.results[c]["out"]).astype(np.float32)
        # v-bias contribution: softmax rows sum to 1, so bv adds the constant
        # row (bv tiled over the 4 q heads) @ Wo_group to every output row
        bv_tiled = np.tile(bv[g * 128 : (g + 1) * 128], NH).astype(np.float64)
        cvec = bv_tiled @ Wo[g * 512 : (g + 1) * 512, :].astype(np.float64)
        out[c // 4] += cvec.astype(np.float32)[None, :]
    return out


def kernel(x, Wq, bq, Wk, bk, Wv, bv, Wo):
    nc = _get_program()
    in_maps = make_in_maps(x, Wq, bq, Wk, bk, Wv, bv, Wo)
    res = run_bass_kernel_spmd(nc, in_maps, list(range(8)))
    return combine_outputs(res, {"bv": bv, "Wo": Wo})


# revision 23
# speedup vs baseline: 1.0269x; 1.0269x over previous
"""Trainium2 Bass kernel for causal GQA self-attention (B=2, T=2048, C=2048,
Hq=16, Hkv=4, d=128, RoPE base 1e6).

Sharding: 8 cores = 2 batches x 4 kv-head groups. Each core computes, for its
(batch b, kv group g): the q/k/v projections restricted to that group (4 q
heads + 1 kv head), RoPE, causal attention, and the partial o_proj
(y_group @ Wo[group rows]). The host sums the 4 partial o_proj outputs per
batch (the all-reduce/unshard step of tensor parallelism).

Device layout notes:
  - x is shipped pre-transposed (xT = x[b].T) and pre-cast to bf16 so the
    contraction dim (C) lands on SBUF partitions for all projection matmuls.
    xT is DMA'd t4-major (512-col t-chunks) so the first projection
    accumulation chain starts ~7us in instead of waiting for the full 8.4MB.
  - q/k are produced transposed (qT/kT = [d, T]); scores are computed
    transposed (S^T = k @ qT, [k, q]) so softmax exp needs no cross-partition
    work and P^T feeds the AV matmul directly (yT = v^T @ P^T), no transposes.
  - projections and attention heads are interleaved (k, q0, v, head0, q1,
    head1, ...) so ACT-engine exp work spreads across the whole kernel
    instead of piling up after all projections.
  - diagonal score/rowsum matmul streams and exp ranges are trimmed to the
    causal lower-triangular region (block granularity).
  - softmax skips max-subtraction (scores are O(1); exp cannot overflow) and
    normalizes at the end: rowsums via a ones-vector matmul, 1/s via DVE
    reciprocal, broadcast via a DRAM round-trip DMA, applied after an eager
    unnormalized PSUM->SBUF eviction of yT.
  - the v bias is folded out entirely: since softmax rows sum to 1, bv
    contributes the constant row bv_tiled @ Wo_g, added on the host.
  - o_proj runs as a dependency-free tail block (all yTn ready), output is
    stored in bf16 to halve the 16.8MB store traffic.
  - DMA issues are spread across engine queues: xT/sscr/stores on Sync,
    weights on ACT, tables on DVE, rope partition-shifts on idle GpSimd.
"""

import numpy as np
import ml_dtypes

import concourse.bass as bass
import concourse.mybir as mybir
from concourse import bacc
from concourse.tile import TileContext
from concourse.bass_utils import run_bass_kernel_spmd
from concourse.masks import make_identity

BF16 = mybir.dt.bfloat16
F32 = mybir.dt.float32

T = 2048
C = 2048
D = 128
NH = 4           # q heads per core
CI = C // 128    # contraction chunks
TC = T // 512    # t chunks of 512
TB = T // 128    # t blocks of 128
SCALE = 1.0 / np.sqrt(D)

_PROGRAM = None


def _ts(i, s):
    return bass.ts(i, s)


def _patch_act_tables():
    """Force every ACT function this kernel uses to resolve to the
    natural_log_exp_and_others table set, so the whole kernel needs exactly
    one ACT_TABLE_LOAD. Returns an undo callable."""
    import concourse.bacc as bacc_mod

    orig = bacc_mod.get_activation_tables
    A = mybir.ActivationFunctionType
    mine = {A.Exp, A.Ln, A.Identity, A.Copy}

    def patched(arch):
        tables = dict(orig(arch))
        for name in tables:
            if name != "natural_log_exp_and_others":
                tables[name] = set(tables[name]) - mine
        return tables

    bacc_mod.get_activation_tables = patched

    def undo():
        bacc_mod.get_activation_tables = orig

    return undo


def _build_program():
    undo = _patch_act_tables()
    try:
        return _build_program_inner()
    finally:
        undo()


def _build_program_inner():
    nc = bacc.Bacc("TRN2", target_bir_lowering=False, debug=False, num_devices=8)

    xT_d = nc.dram_tensor("xT", [C, T], BF16, kind="ExternalInput").ap()
    wq_d = nc.dram_tensor("wq", [C, NH * D], BF16, kind="ExternalInput").ap()
    wk_d = nc.dram_tensor("wk", [C, D], BF16, kind="ExternalInput").ap()
    wv_d = nc.dram_tensor("wv", [C, D], BF16, kind="ExternalInput").ap()
    wo_d = nc.dram_tensor("wo", [NH * D, C], BF16, kind="ExternalInput").ap()
    bq_d = nc.dram_tensor("bq", [D, NH], F32, kind="ExternalInput").ap()
    bk_d = nc.dram_tensor("bk", [D, 1], F32, kind="ExternalInput").ap()
    cos_d = nc.dram_tensor("cosT", [D, T], BF16, kind="ExternalInput").ap()
    sin_d = nc.dram_tensor("sinT", [D, T], BF16, kind="ExternalInput").ap()
    tri_d = nc.dram_tensor("tri", [D, D], BF16, kind="ExternalInput").ap()
    out_d = nc.dram_tensor("out", [T, C], BF16, kind="ExternalOutput").ap()
    # scratch for the per-(head, q-chunk) 1/rowsum broadcast round-trip
    sscr = nc.dram_tensor("sscr", [NH * TC, 512], F32).ap()

    Ident = mybir.ActivationFunctionType.Identity
    Exp = mybir.ActivationFunctionType.Exp

    with TileContext(nc) as tc:
        with (
            tc.tile_pool(name="consts", bufs=1) as consts,
            tc.tile_pool(name="acts", bufs=1) as acts,
        ):
            # ---- resident constants -------------------------------------
            xT_sb = consts.tile([128, CI, T], BF16)
            wq_sb = consts.tile([128, CI, NH * D], BF16)
            wk_sb = consts.tile([128, CI, D], BF16)
            wv_sb = consts.tile([128, CI, D], BF16)
            wo_sb = consts.tile([128, NH, C], BF16)
            bq_sb = consts.tile([128, NH], F32)
            bk_sb = consts.tile([128, 1], F32)
            cos_sb = consts.tile([128, T], BF16)
            sin_sb = consts.tile([128, T], BF16)
            tri_sb = consts.tile([128, 128], BF16)
            ones_sb = consts.tile([128, 1], BF16)
            ident_sb = consts.tile([128, 128], BF16)

            # xT on the Sync queue, t4-major so the first proj chain starts
            # as soon as the first 2.1MB lands. The first 4 chunks are split
            # across partitions into 4 DMAs each, landing on parallel DMA
            # engines, so the very first matmul starts ~3us in, not ~7us.
            for ci in range(4):
                for p4 in range(4):
                    nc.sync.dma_start(
                        out=xT_sb[p4 * 32 : (p4 + 1) * 32, ci, 0:512],
                        in_=xT_d[ci * 128 + p4 * 32 : ci * 128 + (p4 + 1) * 32, 0:512],
                    )
            for t4 in range(TC):
                for ci in range(CI):
                    if t4 == 0 and ci < 4:
                        continue
                    nc.sync.dma_start(
                        out=xT_sb[:, ci, _ts(t4, 512)],
                        in_=xT_d[_ts(ci, 128), _ts(t4, 512)],
                    )
            # k/v weights + biases on the ACT queue (needed first).
            for g4 in range(4):
                wkg = wk_d[g4 * 512 : (g4 + 1) * 512, :]
                nc.scalar.dma_start(
                    out=wk_sb[:, g4 * 4 : (g4 + 1) * 4, :],
                    in_=bass.AP(
                        tensor=wkg.tensor,
                        offset=wkg.offset,
                        ap=[[D, 128], [128 * D, 4], [1, D]],
                    ),
                )
                wvg = wv_d[g4 * 512 : (g4 + 1) * 512, :]
                nc.scalar.dma_start(
                    out=wv_sb[:, g4 * 4 : (g4 + 1) * 4, :],
                    in_=bass.AP(
                        tensor=wvg.tensor,
                        offset=wvg.offset,
                        ap=[[D, 128], [128 * D, 4], [1, D]],
                    ),
                )
            nc.scalar.dma_start(out=bq_sb[:], in_=bq_d[:])
            nc.scalar.dma_start(out=bk_sb[:], in_=bk_d[:])
            # tables + q weights also on the ACT queue, ordered by first use
            # (all issued before the first bias op is queued); wq last, it is
            # not needed until the third chain of the first t4 block
            nc.scalar.dma_start(out=cos_sb[:], in_=cos_d[:])
            nc.scalar.dma_start(out=sin_sb[:], in_=sin_d[:])
            nc.scalar.dma_start(out=tri_sb[:], in_=tri_d[:])
            # wq on the GpSimd queue: its SWDGE issues are slower but rope
            # shifts (the only later GpSimd work) are not needed until the
            # first head, and this keeps the ACT queue free for biases
            for ci in range(CI):
                nc.gpsimd.dma_start(out=wq_sb[:, ci, :], in_=wq_d[_ts(ci, 128), :])
            nc.vector.memset(ones_sb[:], 1.0)
            make_identity(nc, ident_sb[:])

            # ---- persistent activations ---------------------------------
            qT_all = acts.tile([128, NH, T], BF16)   # rotated q^T per head
            kT_all = acts.tile([128, T], BF16)       # rotated k^T
            v_sb = acts.tile([128, TB, D], BF16)     # v in natural [t, d] blocks
            yTn = acts.tile([128, NH, T], BF16)      # normalized y^T per head

            with tc.tile_pool(name="rope", bufs=2) as rope_pool:
                def emit_evict(m, t4, ps, vtp=None):
                    if m == 5:
                        # v: no bias on device (bv folded out on host)
                        vbb = rope_pool.tile([128, 512], BF16, tag="vbb")
                        nc.vector.tensor_copy(vbb[:], ps[:])
                        for j in range(4):
                            tb = t4 * 4 + j
                            pt = vtp.tile([128, 128], BF16)
                            nc.tensor.transpose(
                                pt[:], vbb[:, _ts(j, 128)], ident_sb[:]
                            )
                            nc.vector.tensor_copy(v_sb[:, tb, :], pt[:])
                    else:
                        bias_ap = bq_sb[:, m : m + 1] if m < 4 else bk_sb[:, 0:1]
                        qb = rope_pool.tile([128, 512], BF16, tag="qb")
                        if m == 0:
                            # spread PSUM evictions across engines: the four
                            # zipper chains stop nearly together, and three
                            # serialized ACT biases would stall the pool
                            nc.vector.tensor_scalar_add(qb[:], ps[:], bias_ap)
                        else:
                            nc.scalar.activation(qb[:], ps[:], Ident, bias=bias_ap)
                        # rotate-half partition shift on the idle GpSimd queue
                        sh = rope_pool.tile([128, 512], BF16, tag="sh")
                        nc.gpsimd.dma_start(out=sh[0:64, :], in_=qb[64:128, :])
                        nc.gpsimd.dma_start(out=sh[64:128, :], in_=qb[0:64, :])
                        t1 = rope_pool.tile([128, 512], BF16, tag="t1")
                        nc.vector.tensor_mul(t1[:], qb[:], cos_sb[:, _ts(t4, 512)])
                        nc.vector.tensor_mul(sh[:], sh[:], sin_sb[:, _ts(t4, 512)])
                        dest = (
                            qT_all[:, m, _ts(t4, 512)]
                            if m < 4
                            else kT_all[:, _ts(t4, 512)]
                        )
                        nc.vector.tensor_add(dest, t1[:], sh[:])

                def w_of(m, ci):
                    if m < 4:
                        return wq_sb[:, ci, _ts(m, 128)]
                    if m == 4:
                        return wk_sb[:, ci, :]
                    return wv_sb[:, ci, :]

                with (
                    tc.tile_pool(name="pp4", bufs=6, space="PSUM") as pp4,
                    tc.tile_pool(name="vtp", bufs=2, space="PSUM") as vtp,
                ):
                    # t4-outer: each freshly-landed xT column chunk feeds
                    # four chains (k, v, q0, q1) back to back, so PE consumes
                    # slightly slower than the xT DMA stream arrives
                    for t4 in range(TC):
                        ms = (4, 5, 0, 1)
                        ps = {
                            m: pp4.tile([128, 512], F32, name=f"ps{m}", tag="ps")
                            for m in ms
                        }
                        for ci in range(CI):
                            for m in ms:
                                nc.tensor.matmul(
                                    ps[m][:],
                                    w_of(m, ci),
                                    xT_sb[:, ci, _ts(t4, 512)],
                                    start=(ci == 0),
                                    stop=(ci == CI - 1),
                                )
                        for m in ms:
                            emit_evict(m, t4, ps[m], vtp)

                with (
                    tc.tile_pool(name="pp", bufs=2, space="PSUM") as pp,
                    tc.tile_pool(name="st", bufs=3, space="PSUM") as stp,
                    tc.tile_pool(name="yt", bufs=2, space="PSUM") as ytp,
                    tc.tile_pool(name="rs", bufs=1, space="PSUM") as rsp,
                    tc.tile_pool(name="ptp", bufs=6) as ptp,
                    tc.tile_pool(name="sivb", bufs=2) as sivb,
                    tc.tile_pool(name="sip", bufs=2) as sip,
                    tc.tile_pool(name="ytu", bufs=4) as ytup,
                ):
                    def emit_proj(m):
                        for t4 in range(TC):
                            ps = pp.tile([128, 512], F32)
                            for ci in range(CI):
                                nc.tensor.matmul(
                                    ps[:],
                                    w_of(m, ci),
                                    xT_sb[:, ci, _ts(t4, 512)],
                                    start=(ci == 0),
                                    stop=(ci == CI - 1),
                                )
                            emit_evict(m, t4, ps)

                    def emit_head(h):
                        for qc in (3, 2, 1, 0):
                            yt_ps = ytp.tile([128, 512], F32)
                            rs_ps = rsp.tile([1, 512], F32)
                            nkb = 4 * (qc + 1)
                            for kb in range(nkb):
                                j = kb - 4 * qc  # >=0 on the diagonal group
                                lo = j * 128 if j > 0 else 0
                                st_ps = stp.tile([128, 512], F32)
                                # scores, trimmed to the causal region
                                nc.tensor.matmul(
                                    st_ps[:, lo:512],
                                    kT_all[:, _ts(kb, 128)],
                                    qT_all[:, h, qc * 512 + lo : qc * 512 + 512],
                                    start=True,
                                    stop=True,
                                )
                                pt = ptp.tile([128, 512], BF16, tag="pt")
                                nc.scalar.activation(
                                    pt[:, lo:512], st_ps[:, lo:512], Exp,
                                    scale=SCALE,
                                )
                                if j >= 0:
                                    if j > 0:
                                        nc.vector.memset(pt[:, 0:lo], 0.0)
                                    nc.vector.tensor_mul(
                                        pt[:, lo : lo + 128],
                                        pt[:, lo : lo + 128],
                                        tri_sb[:],
                                    )
                                nc.tensor.matmul(
                                    yt_ps[:],
                                    v_sb[:, kb, :],
                                    pt[:],
                                    start=(kb == 0),
                                    stop=(kb == nkb - 1),
                                )
                                # rowsums: trimmed, except the last block
                                # streams full width so every PSUM region
                                # sees its stop flag
                                rlo = lo if j < 3 else 0
                                nc.tensor.matmul(
                                    rs_ps[0:1, rlo:512],
                                    ones_sb[:],
                                    pt[:, rlo:512],
                                    start=(kb == 0),
                                    stop=(kb == nkb - 1),
                                )
                            # evict yT unnormalized right away (frees the
                            # PSUM bank without waiting on the 1/s chain)
                            ytu = ytup.tile([128, 512], BF16)
                            nc.vector.tensor_copy(ytu[:], yt_ps[:])
                            # 1/s on DVE, then DRAM round-trip broadcast
                            idx = h * TC + qc
                            si = sip.tile([1, 512], F32, tag="si")
                            nc.vector.reciprocal_approx_fast(si[:], rs_ps[:])
                            nc.sync.dma_start(out=sscr[idx : idx + 1, :], in_=si[:])
                            sb = sivb.tile([128, 512], F32)
                            row = sscr[idx : idx + 1, :]
                            bc = bass.AP(
                                tensor=row.tensor,
                                offset=row.offset,
                                ap=[[0, 128]] + row.ap[1:],
                            )
                            nc.sync.dma_start(out=sb[:], in_=bc)
                            # normalize on the idle GpSimd engine: a DVE mul
                            # here would stall the in-order DVE queue on the
                            # broadcast round-trip, delaying the next chunk's
                            # mask work and stalling PE
                            nc.gpsimd.tensor_mul(
                                yTn[:, h, _ts(qc, 512)], ytu[:], sb[:]
                            )

                    emit_head(0)
                    emit_proj(2)
                    # wo on the ACT queue once attention is underway
                    for hh in range(NH):
                        nc.scalar.dma_start(
                            out=wo_sb[:, hh, :], in_=wo_d[_ts(hh, 128), :]
                        )
                    emit_head(1)
                    emit_proj(3)
                    emit_head(2)
                    emit_head(3)

            # ---- o_proj tail: all yTn ready, pure streaming -------------
            with (
                tc.tile_pool(name="po", bufs=6, space="PSUM") as pop,
                tc.tile_pool(name="oe", bufs=6) as oep,
            ):
                # qc-descending ti order: yTn for low qc finishes last, so
                # the first o_proj tiles must not depend on it
                for qcg in (3, 2, 1, 0):
                    for ti in range(4 * qcg, 4 * qcg + 4):
                        for nj in range(TC):
                            ps = pop.tile([128, 512], F32)
                            for h in range(NH):
                                nc.tensor.matmul(
                                    ps[:],
                                    yTn[:, h, _ts(ti, 128)],
                                    wo_sb[:, h, _ts(nj, 512)],
                                    start=(h == 0),
                                    stop=(h == NH - 1),
                                )
                            oe = oep.tile([128, 512], BF16)
                            if (ti * TC + nj) % 2 == 0:
                                nc.vector.tensor_copy(oe[:], ps[:])
                                nc.sync.dma_start(
                                    out=out_d[_ts(ti, 128), _ts(nj, 512)],
                                    in_=oe[:],
                                )
                            else:
                                nc.scalar.copy(oe[:], ps[:])
                                nc.scalar.dma_start(
                                    out=out_d[_ts(ti, 128), _ts(nj, 512)],
                                    in_=oe[:],
                                )

    nc.finalize()
    return nc


def _get_program():
    global _PROGRAM
    if _PROGRAM is None:
        _PROGRAM = _build_program()
    return _PROGRAM


def _rope_tables():
    inv_freq = 1.0 / (1000000.0 ** (np.arange(0, D, 2, dtype=np.float64) / D))
    pos = np.arange(T, dtype=np.float64)
    si = np.outer(pos, inv_freq)                      # [T, D/2]
    cos_h, sin_h = np.cos(si), np.sin(si)
    cos = np.stack([cos_h, cos_h], axis=-1).reshape(T, D)
    sin = np.stack([sin_h, sin_h], axis=-1).reshape(T, D)
    cosT = np.ascontiguousarray(cos.T).astype(np.float32)   # [D, T]
    sinT = np.ascontiguousarray(sin.T).astype(np.float32)
    # rotate-half as a partition shift: sh[i<64]=q[i+64], sh[i>=64]=q[i-64];
    # q_rot = q*cos + sh*sin_signed with the -1 for i<64 baked into the table
    sinT[: D // 2] *= -1.0
    return cosT, sinT


def make_in_maps(x, Wq, bq, Wk, bk, Wv, bv, Wo):
    bf = ml_dtypes.bfloat16
    cosT, sinT = _rope_tables()
    tri = np.triu(np.ones((D, D), dtype=np.float32)).astype(bf)  # [k, q]: q >= k
    in_maps = []
    for b in range(2):
        xT = np.ascontiguousarray(x[b].T).astype(bf)
        for g in range(4):
            in_maps.append(
                {
                    "xT": xT,
                    "wq": np.ascontiguousarray(Wq[:, g * 512 : (g + 1) * 512]).astype(bf),
                    "wk": np.ascontiguousarray(Wk[:, g * 128 : (g + 1) * 128]).astype(bf),
                    "wv": np.ascontiguousarray(Wv[:, g * 128 : (g + 1) * 128]).astype(bf),
                    "wo": np.ascontiguousarray(Wo[g * 512 : (g + 1) * 512, :]).astype(bf),
                    "bq": np.ascontiguousarray(
                        bq[g * 512 : (g + 1) * 512].reshape(NH, D).T
                    ).astype(np.float32),
                    "bk": np.ascontiguousarray(
                        bk[g * 128 : (g + 1) * 128].reshape(D, 1)
                    ).astype(np.float32),
                    "cosT": cosT.astype(bf),
                    "sinT": sinT.astype(bf),
                    "tri": tri,
                }
            )
    return in_maps


def combine_outputs(res, inputs):
    bv, Wo = np.asarray(inputs["bv"]), np.asarray(inputs["Wo"])
    out = np.zeros((2, T, C), dtype=np.float32)
    for c in range(8):
        g = c % 4
        out[c // 4] += np.asarray(res.results[c]["out"]).astype(np.float32)
        # v-bias contribution: softmax rows sum to 1, so bv adds the constant
        # row (bv tiled over the 4 q heads) @ Wo_group to every output row
        bv_tiled = np.tile(bv[g * 128 : (g + 1) * 128], NH).astype(np.float64)
        cvec = bv_tiled @ Wo[g * 512 : (g + 1) * 512, :].astype(np.float64)
        out[c // 4] += cvec.astype(np.float32)[None, :]
    return out


def kernel(x, Wq, bq, Wk, bk, Wv, bv, Wo):
    nc = _get_program()
    in_maps = make_in_maps(x, Wq, bq, Wk, bk, Wv, bv, Wo)
    res = run_bass_kernel_spmd(nc, in_maps, list(range(8)))
    return combine_outputs(res, {"bv": bv, "Wo": Wo})
